# revision 39
# baseline (speedup 1.0000x reference)
"""Chamfer distance loss (per-cluster, bidirectional) on 8 Trainium2 cores.

Problem: points [131072, 3] in 128 equal clusters of 1024. Per cluster c:
  d[i,j] = ||a_i - b_j||^2 ; loss_c = sum_i min_j d + sum_j min_i d
Total = sum of loss_c over clusters 0..126 (the max cluster id is excluded).

Strategy (data-parallel over clusters, 16 clusters/core), single matmul
pass per cluster computing NEGATED distances so mins become maxes:
  - Host packs per cluster two K=13 operand matrices (split-fp16 for
    accuracy); B side negated so PE emits -d directly into PSUM f32.
  - Clusters processed in pairs: 32 matmuls of [128, 512] fill double
    PSUM tiles; the Act (scalar) engine converts each double tile into
    two slots of a batched SBUF f16 tile t16 [128, 16, 1024] (Act is
    the only engine besides DVE that may read PSUM).
  - DVE dir-1 (min over j): one batched j-half fold tree over all 16
    chunk slots (tensor_tensor max, f16 2x_1p mode, in-place halving)
    + one segmented reduce -> rowmax [128, 16] per pair.
  - DVE dir-2 (min over i): fold tree across chunk sub-tiles (clusters
    kept separate via 4D APs) -> r2 [128, 2, 1024] f16; DMA xbar
    transpose puts j on partitions; two more folds + a segmented
    reduce give colmax [128, 16] per pair.
  - Pair 0 runs per-cluster to shorten the pipeline fill; outputs are
    DMA'd out incrementally. Host: loss = -sum of masked row/col maxes.

Notes from HW probing (TRN2):
  - tensor_tensor_reduce compiles but faults the device -> unusable.
  - Two-input ops may read at most ONE operand from PSUM; GpSimd may
    not touch PSUM at all and has no TensorTensor opcode on TRN2.
  - DVE tensor_tensor on packed f16 runs in 2x_1p mode (0.5 cyc/elem);
    tensor_reduce has no fast mode -> fold trees + small final reduce.
  - gpsimd.partition_all_reduce works (attn ucode library) but its
    SBUF traffic steals the DVE's shared ports and drops concurrent
    folds out of 2x mode -- the DMA-transpose path is faster.
  - Matmul output must stay within one PSUM bank (512 f32 cols max).
"""

import numpy as np

C = 128          # clusters
P = 1024         # points per cluster
DIM = 3
K = 13           # augmented contraction dim (split-fp16 rows)
N_CORES = 8
CPC = C // N_CORES   # clusters per core (16)
ICH = P // 128       # i-chunks per cluster (8)

_cache = {}


def _build():
    import concourse.bacc as bacc
    import concourse.mybir as mybir
    from concourse.tile import TileContext

    nc = bacc.Bacc(
        "TRN2", target_bir_lowering=False, debug=False, num_devices=N_CORES)
    f32 = mybir.dt.float32
    f16 = mybir.dt.float16
    mx = mybir.AluOpType.max

    a_d = nc.dram_tensor("a_op", [K, CPC * P], f16, kind="ExternalInput")
    b_d = nc.dram_tensor("b_op", [K, CPC * P], f16, kind="ExternalInput")
    rmax_d = nc.dram_tensor(
        "rowmax", [128, CPC * ICH], f32, kind="ExternalOutput")
    cmax_d = nc.dram_tensor(
        "colmax", [128, CPC * ICH], f32, kind="ExternalOutput")

    with TileContext(nc) as tc:
        with (
            tc.tile_pool(name="const", bufs=1) as cpool,
            tc.tile_pool(name="psum", bufs=2, space="PSUM") as ppool,
            tc.tile_pool(name="tbat", bufs=2) as tpool,
            tc.tile_pool(name="tree", bufs=1) as ypool,
        ):
            a_t = cpool.tile([K, CPC * P], f16)
            b_t = cpool.tile([K, CPC * P], f16)
            # front-load a small first chunk so the first matmul can start
            # as early as possible; the rest in bulk chunks
            splits = [0, 2 * P, 6 * P, 11 * P, CPC * P]
            for q in range(len(splits) - 1):
                lo, hi = splits[q], splits[q + 1]
                nc.sync.dma_start(out=a_t[:, lo:hi], in_=a_d[:, lo:hi])
                nc.sync.dma_start(out=b_t[:, lo:hi], in_=b_d[:, lo:hi])
            rowmax = cpool.tile([128, CPC * ICH], f32)
            colmax = cpool.tile([128, CPC * ICH], f32)

            for cp in range(CPC // 2):
                # process clusters in pairs: one batched t16 tile, larger
                # (fewer) DVE instructions, in-place fold trees
                t16 = tpool.tile([128, 2 * ICH, P], f16, tag="t16")
                for g in range(2):
                    cs = (2 * cp + g) * P
                    for icp in range(ICH // 2):
                        # double PSUM tile (4 banks): one Act copy drains two
                        # i-chunks, amortizing the activation access setup
                        ps = ppool.tile([128, 2 * P], f32, tag="ps")
                        for h in range(2):
                            ic = 2 * icp + h
                            lhsT = a_t[:, cs + ic * 128:cs + (ic + 1) * 128]
                            nc.tensor.matmul(
                                ps[:, h * P:h * P + 512], lhsT,
                                b_t[:, cs:cs + 512], start=True, stop=True)
                            nc.tensor.matmul(
                                ps[:, h * P + 512:(h + 1) * P], lhsT,
                                b_t[:, cs + 512:cs + P], start=True, stop=True)
                        s = g * ICH + 2 * icp
                        nc.scalar.copy(out=t16[:, s:s + 2, :], in_=ps[:])

                # per-pair reductions. Pair 0 is processed per-cluster so the
                # DVE starts after 8 Act copies (shorter pipeline fill);
                # later pairs use one batched tree (fewer instructions).
                # Order: dir-2 folds -> DMA transpose -> dir-1 tree (overlaps
                # the transpose) -> transposed folds + reduces.
                if cp == 0:
                    groups = [(0, ICH), (ICH, ICH)]   # (slot offset, nslots)
                else:
                    groups = [(0, 2 * ICH)]
                for s0, ns in groups:
                    oc = cp * 2 * ICH + s0            # output column base
                    sl = t16[:, s0:s0 + ns, :]
                    g2 = ns // ICH                    # clusters in this group
                    slv = sl.rearrange("p (g s) j -> p g s j", g=g2)

                    # dir-2: fold across chunk sub-tiles (clusters separate)
                    w4 = ypool.tile([128, g2, 4, P], f16, tag=f"w4_{g2}")
                    nc.vector.tensor_tensor(
                        out=w4[:], in0=slv[:, :, 0:4, :],
                        in1=slv[:, :, 4:8, :], op=mx)
                    nc.vector.tensor_tensor(
                        out=w4[:, :, 0:2, :], in0=w4[:, :, 0:2, :],
                        in1=w4[:, :, 2:4, :], op=mx)
                    r2 = ypool.tile([128, g2, P], f16, tag=f"r2_{g2}")
                    nc.vector.tensor_tensor(
                        out=r2[:], in0=w4[:, :, 0, :], in1=w4[:, :, 1, :],
                        op=mx)
                    rt = ypool.tile([128, ns, 128], f16, tag=f"rt_{g2}")
                    nc.sync.dma_start_transpose(
                        out=rt[:], in_=r2[:].rearrange("p g j -> p (g j)"))

                    # dir-1: j-half fold tree (in-place) + segmented reduce
                    y1 = ypool.tile([128, ns, 512], f16, tag=f"y1_{g2}")
                    nc.vector.tensor_tensor(
                        out=y1[:], in0=sl[:, :, 0:512],
                        in1=sl[:, :, 512:1024], op=mx)
                    for hw_ in (256, 128, 64, 32):
                        nc.vector.tensor_tensor(
                            out=y1[:, :, 0:hw_], in0=y1[:, :, 0:hw_],
                            in1=y1[:, :, hw_:2 * hw_], op=mx)
                    # last fold lands in a compact tile so the reduce input
                    # is contiguous (strided reduce inputs cost ~200ns extra)
                    yc = ypool.tile([128, ns, 16], f16, tag=f"yc_{g2}")
                    nc.vector.tensor_tensor(
                        out=yc[:], in0=y1[:, :, 0:16],
                        in1=y1[:, :, 16:32], op=mx)
                    nc.vector.tensor_reduce(
                        out=rowmax[:, oc:oc + ns], in_=yc[:],
                        axis=mybir.AxisListType.X, op=mx)

                    # lane-axis max tail on the transposed tile
                    nc.vector.tensor_tensor(
                        out=rt[:, :, 0:64], in0=rt[:, :, 0:64],
                        in1=rt[:, :, 64:128], op=mx)
                    nc.vector.tensor_tensor(
                        out=rt[:, :, 0:32], in0=rt[:, :, 0:32],
                        in1=rt[:, :, 32:64], op=mx)
                    rc = ypool.tile([128, ns, 16], f16, tag=f"rc_{g2}")
                    nc.vector.tensor_tensor(
                        out=rc[:], in0=rt[:, :, 0:16],
                        in1=rt[:, :, 16:32], op=mx)
                    nc.vector.tensor_reduce(
                        out=colmax[:, oc:oc + ns], in_=rc[:],
                        axis=mybir.AxisListType.X, op=mx)
                    # stream results out as they complete (keeps the final
                    # DMA off the drain tail)
                    nc.sync.dma_start(
                        out=rmax_d[:, oc:oc + ns], in_=rowmax[:, oc:oc + ns])
                    nc.sync.dma_start(
                        out=cmax_d[:, oc:oc + ns], in_=colmax[:, oc:oc + ns])
    nc.compile()
    return nc


def _split(x):
    """fp32 -> (hi, lo) fp16 pair with x ~= hi + lo."""
    hi = x.astype(np.float16)
    lo = (x - hi.astype(np.float32)).astype(np.float16)
    return hi, lo


def _prep(input_points, output_points):
    a = np.ascontiguousarray(input_points, dtype=np.float32).reshape(C, P, DIM)
    b = np.ascontiguousarray(output_points, dtype=np.float32).reshape(C, P, DIM)
    aa = np.einsum("cpd,cpd->cp", a, a).astype(np.float32)
    bb = np.einsum("cpd,cpd->cp", b, b).astype(np.float32)

    at = a.transpose(0, 2, 1)            # [C,3,P]
    bt2 = -2.0 * b.transpose(0, 2, 1)    # [C,3,P]  (B = -2b)
    ah, al = _split(at)
    bh, bl = _split(bt2)
    aah, aal = _split(aa)
    bbh, bbl = _split(bb)

    # -d = sum_k A[k,i] * (-B_orig[k,j]): negate the whole B side.
    a_op = np.empty((C, K, P), np.float16)
    a_op[:, 0:3] = ah
    a_op[:, 3:6] = al
    a_op[:, 6:9] = ah
    a_op[:, 9:11] = 1.0
    a_op[:, 11] = aah
    a_op[:, 12] = aal

    b_op = np.empty((C, K, P), np.float16)
    b_op[:, 0:3] = -bh
    b_op[:, 3:6] = -bh
    b_op[:, 6:9] = -bl
    b_op[:, 9] = -bbh
    b_op[:, 10] = -bbl
    b_op[:, 11:13] = -1.0

    in_maps = []
    for i in range(N_CORES):
        sl = slice(i * CPC, (i + 1) * CPC)
        in_maps.append({
            "a_op": np.ascontiguousarray(
                a_op[sl].transpose(1, 0, 2).reshape(K, CPC * P)),
            "b_op": np.ascontiguousarray(
                b_op[sl].transpose(1, 0, 2).reshape(K, CPC * P)),
        })
    return in_maps


def run(inputs, trace=False, trace_kwargs=None):
    """Returns (loss ndarray shape (), BassKernelResults)."""
    from concourse.bass_utils import run_bass_kernel_spmd

    if "nc" not in _cache:
        _cache["nc"] = _build()
    nc = _cache["nc"]

    in_maps = _prep(inputs["input_points"], inputs["output_points"])
    res = run_bass_kernel_spmd(
        nc, in_maps, list(range(N_CORES)),
        trace=trace, **(trace_kwargs or {}))

    # rowmax[core]: [128, CPC*ICH]; col c*8+t = max_j(-d) for i-chunk t
    # colmax[core]: [CPC, 1024]; row c = max_i(-d) per j
    per_cluster = np.empty(C, np.float64)
    for i in range(N_CORES):
        rm = res.results[i]["rowmax"].astype(np.float64)
        cm = res.results[i]["colmax"].astype(np.float64)
        l1 = rm.reshape(128, CPC, ICH).sum(axis=(0, 2))  # [CPC]
        l2 = cm.reshape(128, CPC, ICH).sum(axis=(0, 2))  # [CPC]
        per_cluster[i * CPC:(i + 1) * CPC] = -(l1 + l2)

    nb = int(np.max(inputs["input_clusters"]))
    mask = np.arange(C) < nb
    total = np.float32(per_cluster[mask].sum())
    return np.array(total, dtype=np.float32), res


def kernel(input_points, input_clusters, output_points, output_clusters):
    loss, _ = run({
        "input_points": input_points,
        "input_clusters": input_clusters,
        "output_points": output_points,
        "output_clusters": output_clusters,
    })
    return loss


# revision 40
# speedup vs baseline: 1.0007x; 1.0007x over previous
"""Chamfer distance loss (per-cluster, bidirectional) on 8 Trainium2 cores.

Problem: points [131072, 3] in 128 equal clusters of 1024. Per cluster c:
  d[i,j] = ||a_i - b_j||^2 ; loss_c = sum_i min_j d + sum_j min_i d
Total = sum of loss_c over clusters 0..126 (the max cluster id is excluded).

Strategy (data-parallel over clusters, 16 clusters/core), single matmul
pass per cluster computing NEGATED distances so mins become maxes:
  - Host packs per cluster two K=13 operand matrices (split-fp16 for
    accuracy); B side negated so PE emits -d directly into PSUM f32.
  - Clusters processed in pairs: 32 matmuls of [128, 512] fill double
    PSUM tiles; the Act (scalar) engine converts each double tile into
    two slots of a batched SBUF f16 tile t16 [128, 16, 1024] (Act is
    the only engine besides DVE that may read PSUM).
  - DVE dir-1 (min over j): one batched j-half fold tree over all 16
    chunk slots (tensor_tensor max, f16 2x_1p mode, in-place halving)
    + one segmented reduce -> rowmax [128, 16] per pair.
  - DVE dir-2 (min over i): fold tree across chunk sub-tiles (clusters
    kept separate via 4D APs) -> r2 [128, 2, 1024] f16; DMA xbar
    transpose puts j on partitions; two more folds + a segmented
    reduce give colmax [128, 16] per pair.
  - Pair 0 runs per-cluster to shorten the pipeline fill; outputs are
    DMA'd out incrementally. Host: loss = -sum of masked row/col maxes.

Notes from HW probing (TRN2):
  - tensor_tensor_reduce compiles but faults the device -> unusable.
  - Two-input ops may read at most ONE operand from PSUM; GpSimd may
    not touch PSUM at all and has no TensorTensor opcode on TRN2.
  - DVE tensor_tensor on packed f16 runs in 2x_1p mode (0.5 cyc/elem);
    tensor_reduce has no fast mode -> fold trees + small final reduce.
  - gpsimd.partition_all_reduce works (attn ucode library) but its
    SBUF traffic steals the DVE's shared ports and drops concurrent
    folds out of 2x mode -- the DMA-transpose path is faster.
  - Matmul output must stay within one PSUM bank (512 f32 cols max).
"""

import numpy as np

C = 128          # clusters
P = 1024         # points per cluster
DIM = 3
K = 13           # augmented contraction dim (split-fp16 rows)
N_CORES = 8
CPC = C // N_CORES   # clusters per core (16)
ICH = P // 128       # i-chunks per cluster (8)

_cache = {}


def _build():
    import concourse.bacc as bacc
    import concourse.mybir as mybir
    from concourse.tile import TileContext

    nc = bacc.Bacc(
        "TRN2", target_bir_lowering=False, debug=False, num_devices=N_CORES)
    f32 = mybir.dt.float32
    f16 = mybir.dt.float16
    mx = mybir.AluOpType.max

    a_d = nc.dram_tensor("a_op", [K, CPC * P], f16, kind="ExternalInput")
    b_d = nc.dram_tensor("b_op", [K, CPC * P], f16, kind="ExternalInput")
    rmax_d = nc.dram_tensor(
        "rowmax", [128, CPC * ICH], f32, kind="ExternalOutput")
    cmax_d = nc.dram_tensor(
        "colmax", [128, CPC * ICH], f32, kind="ExternalOutput")

    with TileContext(nc) as tc:
        with (
            tc.tile_pool(name="const", bufs=1) as cpool,
            tc.tile_pool(name="psum", bufs=2, space="PSUM") as ppool,
            tc.tile_pool(name="tbat", bufs=2) as tpool,
            tc.tile_pool(name="tree", bufs=1) as ypool,
        ):
            a_t = cpool.tile([K, CPC * P], f16)
            b_t = cpool.tile([K, CPC * P], f16)
            # front-load a small first chunk so the first matmul can start
            # as early as possible; the rest in bulk chunks
            splits = [0, 2 * P, 6 * P, 11 * P, CPC * P]
            for q in range(len(splits) - 1):
                lo, hi = splits[q], splits[q + 1]
                nc.sync.dma_start(out=a_t[:, lo:hi], in_=a_d[:, lo:hi])
                nc.sync.dma_start(out=b_t[:, lo:hi], in_=b_d[:, lo:hi])
            rowmax = cpool.tile([128, CPC * ICH], f32)
            colmax = cpool.tile([128, CPC * ICH], f32)

            for cp in range(CPC // 2):
                # process clusters in pairs: one batched t16 tile, larger
                # (fewer) DVE instructions, in-place fold trees
                t16 = tpool.tile([128, 2 * ICH, P], f16, tag="t16")
                for g in range(2):
                    cs = (2 * cp + g) * P
                    for icp in range(ICH // 2):
                        # double PSUM tile (4 banks): one Act copy drains two
                        # i-chunks, amortizing the activation access setup
                        ps = ppool.tile([128, 2 * P], f32, tag="ps")
                        for h in range(2):
                            ic = 2 * icp + h
                            lhsT = a_t[:, cs + ic * 128:cs + (ic + 1) * 128]
                            nc.tensor.matmul(
                                ps[:, h * P:h * P + 512], lhsT,
                                b_t[:, cs:cs + 512], start=True, stop=True)
                            nc.tensor.matmul(
                                ps[:, h * P + 512:(h + 1) * P], lhsT,
                                b_t[:, cs + 512:cs + P], start=True, stop=True)
                        s = g * ICH + 2 * icp
                        nc.scalar.copy(out=t16[:, s:s + 2, :], in_=ps[:])

                # per-pair reductions. Pair 0 is processed per-cluster so the
                # DVE starts after 8 Act copies (shorter pipeline fill);
                # later pairs use one batched tree (fewer instructions).
                # Order: dir-2 folds -> DMA transpose -> dir-1 tree (overlaps
                # the transpose) -> transposed folds + reduces.
                if cp == 0:
                    groups = [(0, ICH), (ICH, ICH)]   # (slot offset, nslots)
                else:
                    groups = [(0, 2 * ICH)]
                for s0, ns in groups:
                    oc = cp * 2 * ICH + s0            # output column base
                    sl = t16[:, s0:s0 + ns, :]
                    g2 = ns // ICH                    # clusters in this group
                    slv = sl.rearrange("p (g s) j -> p g s j", g=g2)

                    # dir-2: fold across chunk sub-tiles (clusters separate)
                    w4 = ypool.tile([128, g2, 4, P], f16, tag=f"w4_{g2}")
                    nc.vector.tensor_tensor(
                        out=w4[:], in0=slv[:, :, 0:4, :],
                        in1=slv[:, :, 4:8, :], op=mx)
                    nc.vector.tensor_tensor(
                        out=w4[:, :, 0:2, :], in0=w4[:, :, 0:2, :],
                        in1=w4[:, :, 2:4, :], op=mx)
                    r2 = ypool.tile([128, g2, P], f16, tag=f"r2_{g2}")
                    nc.vector.tensor_tensor(
                        out=r2[:], in0=w4[:, :, 0, :], in1=w4[:, :, 1, :],
                        op=mx)
                    rt = ypool.tile([128, ns, 128], f16, tag=f"rt_{g2}")
                    nc.sync.dma_start_transpose(
                        out=rt[:], in_=r2[:].rearrange("p g j -> p (g j)"))

                    # dir-1: j-half fold tree (in-place) + segmented reduce
                    y1 = ypool.tile([128, ns, 512], f16, tag=f"y1_{g2}")
                    nc.vector.tensor_tensor(
                        out=y1[:], in0=sl[:, :, 0:512],
                        in1=sl[:, :, 512:1024], op=mx)
                    for hw_ in (256, 128, 64, 32, 16):
                        nc.vector.tensor_tensor(
                            out=y1[:, :, 0:hw_], in0=y1[:, :, 0:hw_],
                            in1=y1[:, :, hw_:2 * hw_], op=mx)
                    nc.vector.tensor_reduce(
                        out=rowmax[:, oc:oc + ns], in_=y1[:, :, 0:16],
                        axis=mybir.AxisListType.X, op=mx)

                    # lane-axis max tail on the transposed tile
                    nc.vector.tensor_tensor(
                        out=rt[:, :, 0:64], in0=rt[:, :, 0:64],
                        in1=rt[:, :, 64:128], op=mx)
                    nc.vector.tensor_tensor(
                        out=rt[:, :, 0:32], in0=rt[:, :, 0:32],
                        in1=rt[:, :, 32:64], op=mx)
                    nc.vector.tensor_tensor(
                        out=rt[:, :, 0:16], in0=rt[:, :, 0:16],
                        in1=rt[:, :, 16:32], op=mx)
                    nc.vector.tensor_reduce(
                        out=colmax[:, oc:oc + ns], in_=rt[:, :, 0:16],
                        axis=mybir.AxisListType.X, op=mx)
                    # stream results out as they complete (keeps the final
                    # DMA off the drain tail)
                    nc.sync.dma_start(
                        out=rmax_d[:, oc:oc + ns], in_=rowmax[:, oc:oc + ns])
                    nc.sync.dma_start(
                        out=cmax_d[:, oc:oc + ns], in_=colmax[:, oc:oc + ns])
    nc.compile()
    return nc


def _split(x):
    """fp32 -> (hi, lo) fp16 pair with x ~= hi + lo."""
    hi = x.astype(np.float16)
    lo = (x - hi.astype(np.float32)).astype(np.float16)
    return hi, lo


def _prep(input_points, output_points):
    a = np.ascontiguousarray(input_points, dtype=np.float32).reshape(C, P, DIM)
    b = np.ascontiguousarray(output_points, dtype=np.float32).reshape(C, P, DIM)
    aa = np.einsum("cpd,cpd->cp", a, a).astype(np.float32)
    bb = np.einsum("cpd,cpd->cp", b, b).astype(np.float32)

    at = a.transpose(0, 2, 1)            # [C,3,P]
    bt2 = -2.0 * b.transpose(0, 2, 1)    # [C,3,P]  (B = -2b)
    ah, al = _split(at)
    bh, bl = _split(bt2)
    aah, aal = _split(aa)
    bbh, bbl = _split(bb)

    # -d = sum_k A[k,i] * (-B_orig[k,j]): negate the whole B side.
    a_op = np.empty((C, K, P), np.float16)
    a_op[:, 0:3] = ah
    a_op[:, 3:6] = al
    a_op[:, 6:9] = ah
    a_op[:, 9:11] = 1.0
    a_op[:, 11] = aah
    a_op[:, 12] = aal

    b_op = np.empty((C, K, P), np.float16)
    b_op[:, 0:3] = -bh
    b_op[:, 3:6] = -bh
    b_op[:, 6:9] = -bl
    b_op[:, 9] = -bbh
    b_op[:, 10] = -bbl
    b_op[:, 11:13] = -1.0

    in_maps = []
    for i in range(N_CORES):
        sl = slice(i * CPC, (i + 1) * CPC)
        in_maps.append({
            "a_op": np.ascontiguousarray(
                a_op[sl].transpose(1, 0, 2).reshape(K, CPC * P)),
            "b_op": np.ascontiguousarray(
                b_op[sl].transpose(1, 0, 2).reshape(K, CPC * P)),
        })
    return in_maps


def run(inputs, trace=False, trace_kwargs=None):
    """Returns (loss ndarray shape (), BassKernelResults)."""
    from concourse.bass_utils import run_bass_kernel_spmd

    if "nc" not in _cache:
        _cache["nc"] = _build()
    nc = _cache["nc"]

    in_maps = _prep(inputs["input_points"], inputs["output_points"])
    res = run_bass_kernel_spmd(
        nc, in_maps, list(range(N_CORES)),
        trace=trace, **(trace_kwargs or {}))

    # rowmax[core]: [128, CPC*ICH]; col c*8+t = max_j(-d) for i-chunk t
    # colmax[core]: [CPC, 1024]; row c = max_i(-d) per j
    per_cluster = np.empty(C, np.float64)
    for i in range(N_CORES):
        rm = res.results[i]["rowmax"].astype(np.float64)
        cm = res.results[i]["colmax"].astype(np.float64)
        l1 = rm.reshape(128, CPC, ICH).sum(axis=(0, 2))  # [CPC]
        l2 = cm.reshape(128, CPC, ICH).sum(axis=(0, 2))  # [CPC]
        per_cluster[i * CPC:(i + 1) * CPC] = -(l1 + l2)

    nb = int(np.max(inputs["input_clusters"]))
    mask = np.arange(C) < nb
    total = np.float32(per_cluster[mask].sum())
    return np.array(total, dtype=np.float32), res


def kernel(input_points, input_clusters, output_points, output_clusters):
    loss, _ = run({
        "input_points": input_points,
        "input_clusters": input_clusters,
        "output_points": output_points,
        "output_clusters": output_clusters,
    })
    return loss
